# revision 16
# baseline (speedup 1.0000x reference)
"""Chord sparse-attention kernel for 8 TRN2 NeuronCores (Bass/Tile).

Problem (hardcoded): B=2, N=4096, E=256, H=512, N_W=12 layers, L=13 links,
offsets [0,1,2,4,...,2048].

Sharding: core c -> (batch b=c//4, quarter q=c%4) owns tokens
[1024q, 1024q+1024) of batch b for all per-token MLP work. Chord mixing is
replicated within each 4-core batch group (stage 1) after two AllGathers
(V post-gMLP, W sparse weights).

Mixing formulation: for each 128-row block r, the new V block is a sum of 6
PE matmuls: a [256,128] "band pair" lhsT (links with offset<=128, spanning
source chunks r and r+1) plus 4 diagonal lhsT matrices (offsets 256, 512,
1024, 2048 reading chunks r+2, r+4, r+8, r+16), accumulated in PSUM. The
band/diag matrices are materialized by scatter-writing W values onto
diagonals of zeroed HBM buffers with strided access patterns (HBM is flat,
so "free offset linear in partition index" is expressible), then DMA'd back
as regular tiles.
"""

import os
import sys
import time

sys.path.insert(0, "/opt/trn_rl_repo")
os.environ.setdefault("JAX_PLATFORMS", "cpu,axon")

import numpy as np

B, N, E, H = 2, 4096, 256, 512
NW = 12  # mixing layers
NL = 13  # chord links
OFFS = [0] + [1 << k for k in range(NL - 1)]
NCORES = 8
NQ = 4  # cores (quarters) per batch
QT = N // NQ  # tokens per core = 1024
NB = N // 128  # 32 blocks per batch
QB = NB // NQ  # 8 blocks per quarter

F32 = None  # set in _build
BF16 = None

_CACHE = {}


def _host_prep(V, data, gW1, gb1, gW2, gb2, fsW1, fsb1, fsW2, fsb2):
    """Build per-core input maps (host-side slicing, transposes, bf16 casts)."""
    import ml_dtypes

    bf16 = ml_dtypes.bfloat16
    f32 = np.float32

    def bfc(x):
        return np.ascontiguousarray(x.astype(bf16))

    # weights, replicated (same for every core)
    gW1_t = bfc(np.asarray(gW1, f32).reshape(2, 128, H).transpose(1, 0, 2))
    gW2_t = bfc(np.asarray(gW2, f32).reshape(4, 128, E).transpose(1, 0, 2))
    fsW1_t = bfc(np.asarray(fsW1, f32).reshape(NW, 2, 128, H).transpose(2, 1, 0, 3))
    fsW2_t = bfc(np.asarray(fsW2, f32).reshape(NW, 4, 128, NL).transpose(2, 1, 0, 3))
    gb1_t = np.ascontiguousarray(np.asarray(gb1, f32).reshape(4, 128).T)  # [128,4]
    fsb1_t = np.ascontiguousarray(
        np.asarray(fsb1, f32).reshape(NW, 4, 128).transpose(2, 0, 1)
    )  # [128,NW,4]
    gb2b = np.tile(np.asarray(gb2, f32)[None, :], (128, 1))  # [128,256]
    fsb2p = np.asarray(fsb2, f32).copy()
    fsb2p[:, 0] += 1.0  # fold the residual (+V) into the off=0 link weight
    fsb2b = np.tile(fsb2p[:, None, :], (1, 128, 1)).transpose(1, 0, 2)  # [128,NW,NL]
    fsb2b = np.ascontiguousarray(fsb2b)

    common = dict(
        gW1=gW1_t, gW2=gW2_t, fsW1=fsW1_t, fsW2=fsW2_t,
        gb1=gb1_t, fsb1=fsb1_t, gb2b=gb2b, fsb2b=fsb2b,
    )

    Vf = np.asarray(V, f32)
    Df = np.asarray(data, f32)
    in_maps = []
    for c in range(NCORES):
        b, q = c // NQ, c % NQ
        sl = slice(QT * q, QT * (q + 1))
        # [E, QT] -> [128, 2, QT] with e = 128*ec + p
        VT = bfc(Vf[b, sl, :].T.reshape(2, 128, QT).transpose(1, 0, 2))
        DT = bfc(Df[b, sl, :].T.reshape(2, 128, QT).transpose(1, 0, 2))
        in_maps.append(dict(VT=VT, dataT=DT, **common))
    return in_maps


def _build():
    import concourse.bacc as bacc
    import concourse.bass as bass
    import concourse.mybir as mybir
    import concourse.tile as tile

    global F32, BF16
    F32 = mybir.dt.float32
    BF16 = mybir.dt.bfloat16
    AF = mybir.ActivationFunctionType

    nc = bacc.Bacc("TRN2", target_bir_lowering=False, debug=False)

    # ---- I/O declarations (shapes match _host_prep) ----
    VT_d = nc.dram_tensor("VT", [128, 2, QT], BF16, kind="ExternalInput")
    DT_d = nc.dram_tensor("dataT", [128, 2, QT], BF16, kind="ExternalInput")
    gW1_d = nc.dram_tensor("gW1", [128, 2, H], BF16, kind="ExternalInput")
    gW2_d = nc.dram_tensor("gW2", [128, 4, E], BF16, kind="ExternalInput")
    fsW1_d = nc.dram_tensor("fsW1", [128, 2, NW, H], BF16, kind="ExternalInput")
    fsW2_d = nc.dram_tensor("fsW2", [128, 4, NW, NL], BF16, kind="ExternalInput")
    gb1_d = nc.dram_tensor("gb1", [128, 4], F32, kind="ExternalInput")
    fsb1_d = nc.dram_tensor("fsb1", [128, NW, 4], F32, kind="ExternalInput")
    gb2b_d = nc.dram_tensor("gb2b", [128, E], F32, kind="ExternalInput")
    fsb2b_d = nc.dram_tensor("fsb2b", [128, NW, NL], F32, kind="ExternalInput")
    out_d = nc.dram_tensor("out", [128, NB, E], F32, kind="ExternalOutput")

    NREG = 3  # band-matrix HBM regions (rotated across layers)
    BP_ELEMS = NB * 256 * 128  # band-pair region elems per batch
    D_ELEMS = NB * 4 * 128 * 128

    groups = [(0, 0, 1, 3), (3, 4, 4, 2), (5, 16, 16, 2), (7, 64, 64, 2)]
    # (link_start, off_start, off_step, count) for band-pair scatter

    with tile.TileContext(nc) as tc:
        with (
            tc.tile_pool(name="wpool", bufs=1) as wp,
            tc.tile_pool(name="stage", bufs=2) as stage,
            tc.tile_pool(name="vpool", bufs=1) as vp,
            tc.tile_pool(name="mats", bufs=3) as mats,
            tc.tile_pool(name="psA", bufs=2, space="PSUM") as psA,
            tc.tile_pool(name="psB", bufs=2, space="PSUM") as psB,
            tc.tile_pool(name="psM", bufs=4, space="PSUM") as psM,
            tc.tile_pool(name="dram", bufs=1, space="DRAM") as dram,
        ):
            # ---- persistent SBUF weights ----
            gW1 = wp.tile([128, 2, H], BF16, tag="gW1")
            gW2 = wp.tile([128, 4, E], BF16, tag="gW2")
            fsW1 = wp.tile([128, 2, NW, H], BF16, tag="fsW1")
            fsW2 = wp.tile([128, 4, NW, NL], BF16, tag="fsW2")
            gb1 = wp.tile([128, 4], F32, tag="gb1")
            fsb1 = wp.tile([128, NW, 4], F32, tag="fsb1")
            gb2b = wp.tile([128, E], F32, tag="gb2b")
            fsb2b = wp.tile([128, NW, NL], F32, tag="fsb2b")
            for t, d in [(gW1, gW1_d), (gW2, gW2_d), (fsW1, fsW1_d), (fsW2, fsW2_d),
                         (gb1, gb1_d), (fsb1, fsb1_d), (gb2b, gb2b_d), (fsb2b, fsb2b_d)]:
                nc.sync.dma_start(t[:], d[:])
            VT = wp.tile([128, 2, QT], BF16, tag="VT")
            DT = wp.tile([128, 2, QT], BF16, tag="DT")
            nc.sync.dma_start(VT[:], VT_d[:])
            nc.sync.dma_start(DT[:], DT_d[:])

            # ---- HBM scratch ----
            bp_reg = [dram.tile([BP_ELEMS], BF16, tag=f"bp{i}", name=f"bp{i}")
                      for i in range(NREG)]
            d_reg = [dram.tile([D_ELEMS], BF16, tag=f"dg{i}", name=f"dg{i}")
                     for i in range(NREG)]
            cinV = dram.tile([QT * E], BF16, tag="cinV")
            coutV = dram.tile([N * E], BF16, tag="coutV")
            cinW = dram.tile([NW * QB * 128 * NL], BF16, tag="cinW")
            coutW = dram.tile([NQ * NW * QB * 128 * NL], BF16, tag="coutW")

            def fap(t, offset, dims):
                a = t.tensor.ap()
                return bass.AP(a.tensor, offset, dims)

            def squeeze(ap):
                dims = [list(d) for d in ap.ap]
                kept = [dims[0]] + [d for d in dims[1:] if d[1] > 1]
                return bass.AP(ap.tensor, ap.offset, kept)

            # ---- zero the band regions (scatters only ever touch diagonals) ----
            zsb = stage.tile([128, 4096], BF16, tag="zero")
            nc.vector.memset(zsb[:], 0.0)
            ZC = 128 * 4096
            for reg, nelem in [(b, BP_ELEMS) for b in bp_reg] + [(d, D_ELEMS) for d in d_reg]:
                for o in range(0, nelem, ZC):
                    nc.sync.dma_start(fap(reg, o, [[4096, 128], [1, 4096]]), zsb[:])

            # ---- g MLP: V1q = gelu(V @ gW1 + gb1) @ gW2 + gb2 (own tokens) ----
            def mlp_hidden(rhsT, W1, b1col):
                """hidT [128, 4, QT] bf16 = gelu(W1.T @ rhsT + b1)."""
                hidT = stage.tile([128, 4, QT], BF16, tag="hidT")
                for ht in range(4):
                    for ns in range(0, QT, 512):
                        ps = psA.tile([128, 512], F32, tag="psA")
                        for ec in range(2):
                            nc.tensor.matmul(
                                ps[:],
                                W1[:, ec, ht * 128:(ht + 1) * 128],
                                rhsT[:, ec, ns:ns + 512],
                                start=(ec == 0), stop=(ec == 1),
                            )
                        nc.scalar.activation(
                            hidT[:, ht, ns:ns + 512], ps[:], AF.Gelu,
                            bias=b1col[:, ht:ht + 1],
                        )
                return hidT

            hidT = mlp_hidden(VT, gW1, gb1)
            V1q = stage.tile([128, QB, E], BF16, tag="V1q")
            for nb in range(QB):
                ps = psM.tile([128, E], F32, tag="psM")
                for ht in range(4):
                    nc.tensor.matmul(
                        ps[:], hidT[:, ht, nb * 128:(nb + 1) * 128], gW2[:, ht, :],
                        start=(ht == 0), stop=(ht == 3),
                    )
                nc.vector.tensor_add(V1q[:, nb, :], ps[:], gb2b[:])

            # AllGather V1 within each batch group -> full-batch V
            nc.sync.dma_start(
                fap(cinV, 0, [[E, 128], [128 * E, QB], [1, E]]), V1q[:]
            )
            nc.gpsimd.collective_compute(
                "AllGather", mybir.AluOpType.bypass,
                replica_groups=[[0, 1, 2, 3], [4, 5, 6, 7]],
                ins=[cinV.opt()], outs=[coutV.opt()],
            )
            Vcur = vp.tile([128, NB, E], BF16, tag="Vping")
            Vnxt = vp.tile([128, NB, E], BF16, tag="Vpong")
            nc.sync.dma_start(
                Vcur[:], fap(coutV, 0, [[E, 128], [128 * E, NB], [1, E]])
            )

            # ---- fs MLPs: W[k] for own tokens, all layers ----
            # Wmine layout [128 p, QB rr, NW k, NL l] so the cinW DMA merges
            # (k, l) into one contiguous dim (3-dim DMA AP limit).
            Wmine = wp.tile([128, QB, NW, NL], BF16, tag="Wmine")
            for k in range(NW):
                hk = mlp_hidden(DT, fsW1[:, :, k, :], fsb1[:, k, :])
                for nb in range(QB):
                    ps = psB.tile([128, NL], F32, tag="psB")
                    for ht in range(4):
                        nc.tensor.matmul(
                            ps[:], hk[:, ht, nb * 128:(nb + 1) * 128],
                            fsW2[:, ht, k, :],
                            start=(ht == 0), stop=(ht == 3),
                        )
                    nc.vector.tensor_add(Wmine[:, nb, k, :], ps[:], fsb2b[:, k, :])

            # AllGather W within batch group. cinW flat layout (rr, p, k, l).
            nc.sync.dma_start(
                fap(cinW, 0,
                    [[NW * NL, 128], [128 * NW * NL, QB], [1, NW * NL]]),
                Wmine[:],
            )
            nc.gpsimd.collective_compute(
                "AllGather", mybir.AluOpType.bypass,
                replica_groups=[[0, 1, 2, 3], [4, 5, 6, 7]],
                ins=[cinW.opt()], outs=[coutW.opt()],
            )
            # coutW flat layout (qq, rr, p, k, l); (qq, rr) merge to one dim.
            Wall = wp.tile([128, NW, NB, NL], BF16, tag="Wall")
            for k in range(NW):
                nc.sync.dma_start(
                    squeeze(Wall[:, k, :, :]),
                    fap(coutW, k * NL,
                        [[NW * NL, 128], [128 * NW * NL, NB], [1, NL]]),
                )

            # ---- scatter W onto band-matrix diagonals (one DMA per link;
            # strided dest costs an extra unit dim, so max 2 iter dims) ----
            def scatter_layer(k):
                bp, dg = bp_reg[k % NREG], d_reg[k % NREG]
                for l in range(9):  # offsets 0..128 -> band-pair matrices
                    nc.sync.dma_start(
                        fap(bp, OFFS[l] * 128, [[129, 128], [32768, NB]]),
                        squeeze(Wall[:, k, :, l:l + 1]),
                    )
                for j in range(4):  # offsets 256..2048 -> diag matrices
                    nc.sync.dma_start(
                        fap(dg, j * 16384, [[129, 128], [4 * 16384, NB]]),
                        squeeze(Wall[:, k, :, 9 + j:10 + j]),
                    )

            # ---- mixing layers ----
            outF = vp.tile([128, NB, E], F32, tag="outF")
            for k in range(NW):
                scatter_layer(k)
                bp, dg = bp_reg[k % NREG], d_reg[k % NREG]
                last = k == NW - 1
                for qq in range(NQ):
                    bsb = mats.tile([128, QB * 2, 128], BF16, tag="bsb")
                    dsb = mats.tile([128, QB * 4, 128], BF16, tag="dsb")
                    nc.sync.dma_start(
                        bsb[:],
                        fap(bp, qq * QB * 32768,
                            [[128, 128], [16384, QB * 2], [1, 128]]),
                    )
                    nc.sync.dma_start(
                        dsb[:],
                        fap(dg, qq * QB * 65536,
                            [[128, 128], [16384, QB * 4], [1, 128]]),
                    )
                    for rr in range(QB):
                        r = qq * QB + rr
                        ps = psM.tile([128, E], F32, tag="psM")
                        nc.tensor.matmul(ps[:], bsb[:, rr * 2, :],
                                         Vcur[:, r, :], start=True, stop=False)
                        nc.tensor.matmul(ps[:], bsb[:, rr * 2 + 1, :],
                                         Vcur[:, (r + 1) % NB, :],
                                         start=False, stop=False)
                        for j, cc in enumerate([2, 4, 8, 16]):
                            nc.tensor.matmul(ps[:], dsb[:, rr * 4 + j, :],
                                             Vcur[:, (r + cc) % NB, :],
                                             start=False, stop=(j == 3))
                        dst = outF[:, r, :] if last else Vnxt[:, r, :]
                        if r % 2 == 0:
                            nc.scalar.copy(dst, ps[:])
                        else:
                            nc.vector.tensor_copy(dst, ps[:])
                Vcur, Vnxt = Vnxt, Vcur

            nc.sync.dma_start(out_d[:], outF[:])
    nc.compile()
    return nc


def _get_compiled():
    if "nc" not in _CACHE:
        _CACHE["nc"] = _build()
    return _CACHE["nc"]


def kernel(**inputs) -> np.ndarray:
    from concourse import bass_utils

    in_maps = _host_prep(
        inputs["V"], inputs["data"], inputs["gW1"], inputs["gb1"], inputs["gW2"],
        inputs["gb2"], inputs["fsW1"], inputs["fsb1"], inputs["fsW2"],
        inputs["fsb2"],
    )
    nc = _get_compiled()
    res = bass_utils.run_bass_kernel_spmd(nc, in_maps, core_ids=list(range(NCORES)))
    out = np.empty((B, N, E), np.float32)
    for b in range(B):
        o = res.results[b * NQ]["out"]  # [128, NB, E], n = 128*nb + p
        out[b] = o.transpose(1, 0, 2).reshape(N, E)
    return out


# revision 18
# speedup vs baseline: 3.7262x; 3.7262x over previous
"""Chord sparse-attention kernel for 8 TRN2 NeuronCores (Bass/Tile).

Problem (hardcoded): B=2, N=4096, E=256, H=512, N_W=12 layers, L=13 links,
offsets [0,1,2,4,...,2048].

Sharding: core c -> (batch b=c//4, quarter q=c%4) owns tokens
[1024q, 1024q+1024) of batch b for all per-token MLP work. Chord mixing is
replicated within each 4-core batch group (stage 1) after two AllGathers
(V post-gMLP, W sparse weights).

Mixing formulation: for each 128-row block r, the new V block is a sum of 6
PE matmuls: a [256,128] "band pair" lhsT (links with offset<=128, spanning
source chunks r and r+1) plus 4 diagonal lhsT matrices (offsets 256, 512,
1024, 2048 reading chunks r+2, r+4, r+8, r+16), accumulated in PSUM. The
band/diag matrices are materialized by scatter-writing W values onto
diagonals of zeroed HBM buffers with strided access patterns (HBM is flat,
so "free offset linear in partition index" is expressible), then DMA'd back
as regular tiles.
"""

import os
import sys
import time

sys.path.insert(0, "/opt/trn_rl_repo")
os.environ.setdefault("JAX_PLATFORMS", "cpu,axon")

import numpy as np

B, N, E, H = 2, 4096, 256, 512
NW = 12  # mixing layers
NL = 13  # chord links
OFFS = [0] + [1 << k for k in range(NL - 1)]
NCORES = 8
NQ = 4  # cores (quarters) per batch
QT = N // NQ  # tokens per core = 1024
NB = N // 128  # 32 blocks per batch
QB = NB // NQ  # 8 blocks per quarter

F32 = None  # set in _build
BF16 = None

_CACHE = {}


def _host_prep(V, data, gW1, gb1, gW2, gb2, fsW1, fsb1, fsW2, fsb2):
    """Build per-core input maps (host-side slicing, transposes, bf16 casts)."""
    import ml_dtypes

    bf16 = ml_dtypes.bfloat16
    f32 = np.float32

    def bfc(x):
        return np.ascontiguousarray(x.astype(bf16))

    # weights, replicated (same for every core)
    gW1_t = bfc(np.asarray(gW1, f32).reshape(2, 128, H).transpose(1, 0, 2))
    gW2_t = bfc(np.asarray(gW2, f32).reshape(4, 128, E).transpose(1, 0, 2))
    fsW1_t = bfc(np.asarray(fsW1, f32).reshape(NW, 2, 128, H).transpose(2, 1, 0, 3))
    fsW2_t = bfc(np.asarray(fsW2, f32).reshape(NW, 4, 128, NL).transpose(2, 1, 0, 3))
    gb1_t = np.ascontiguousarray(np.asarray(gb1, f32).reshape(4, 128).T)  # [128,4]
    fsb1_t = np.ascontiguousarray(
        np.asarray(fsb1, f32).reshape(NW, 4, 128).transpose(2, 0, 1)
    )  # [128,NW,4]
    gb2b = np.tile(np.asarray(gb2, f32)[None, :], (128, 1))  # [128,256]
    fsb2p = np.asarray(fsb2, f32).copy()
    fsb2p[:, 0] += 1.0  # fold the residual (+V) into the off=0 link weight
    fsb2b = np.tile(fsb2p[:, None, :], (1, 128, 1)).transpose(1, 0, 2)  # [128,NW,NL]
    fsb2b = np.ascontiguousarray(fsb2b)

    common = dict(
        gW1=gW1_t, gW2=gW2_t, fsW1=fsW1_t, fsW2=fsW2_t,
        gb1=gb1_t, fsb1=fsb1_t, gb2b=gb2b, fsb2b=fsb2b,
    )

    Vf = np.asarray(V, f32)
    Df = np.asarray(data, f32)
    in_maps = []
    for c in range(NCORES):
        b, q = c // NQ, c % NQ
        sl = slice(QT * q, QT * (q + 1))
        # [E, QT] -> [128, 2, QT] with e = 128*ec + p
        VT = bfc(Vf[b, sl, :].T.reshape(2, 128, QT).transpose(1, 0, 2))
        DT = bfc(Df[b, sl, :].T.reshape(2, 128, QT).transpose(1, 0, 2))
        in_maps.append(dict(VT=VT, dataT=DT, **common))
    return in_maps


def _build():
    import concourse.bacc as bacc
    import concourse.bass as bass
    import concourse.mybir as mybir
    import concourse.tile as tile

    global F32, BF16
    F32 = mybir.dt.float32
    BF16 = mybir.dt.bfloat16
    AF = mybir.ActivationFunctionType

    nc = bacc.Bacc("TRN2", target_bir_lowering=False, debug=False)

    # ---- I/O declarations (shapes match _host_prep) ----
    VT_d = nc.dram_tensor("VT", [128, 2, QT], BF16, kind="ExternalInput")
    DT_d = nc.dram_tensor("dataT", [128, 2, QT], BF16, kind="ExternalInput")
    gW1_d = nc.dram_tensor("gW1", [128, 2, H], BF16, kind="ExternalInput")
    gW2_d = nc.dram_tensor("gW2", [128, 4, E], BF16, kind="ExternalInput")
    fsW1_d = nc.dram_tensor("fsW1", [128, 2, NW, H], BF16, kind="ExternalInput")
    fsW2_d = nc.dram_tensor("fsW2", [128, 4, NW, NL], BF16, kind="ExternalInput")
    gb1_d = nc.dram_tensor("gb1", [128, 4], F32, kind="ExternalInput")
    fsb1_d = nc.dram_tensor("fsb1", [128, NW, 4], F32, kind="ExternalInput")
    gb2b_d = nc.dram_tensor("gb2b", [128, E], F32, kind="ExternalInput")
    fsb2b_d = nc.dram_tensor("fsb2b", [128, NW, NL], F32, kind="ExternalInput")
    out_d = nc.dram_tensor("out", [128, NB, E], F32, kind="ExternalOutput")

    NREG = 3  # band-matrix HBM regions (rotated across layers)
    BP_ELEMS = NB * 256 * 128  # band-pair region elems per batch
    D_ELEMS = NB * 4 * 128 * 128

    groups = [(0, 0, 1, 3), (3, 4, 4, 2), (5, 16, 16, 2), (7, 64, 64, 2)]
    # (link_start, off_start, off_step, count) for band-pair scatter

    with tile.TileContext(nc) as tc:
        with (
            tc.tile_pool(name="wpool", bufs=1) as wp,
            tc.tile_pool(name="stage", bufs=2) as stage,
            tc.tile_pool(name="vpool", bufs=1) as vp,
            tc.tile_pool(name="mats", bufs=3) as mats,
            tc.tile_pool(name="psA", bufs=2, space="PSUM") as psA,
            tc.tile_pool(name="psB", bufs=2, space="PSUM") as psB,
            tc.tile_pool(name="psM", bufs=4, space="PSUM") as psM,
            tc.tile_pool(name="dram", bufs=1, space="DRAM") as dram,
        ):
            # ---- persistent SBUF weights ----
            gW1 = wp.tile([128, 2, H], BF16, tag="gW1")
            gW2 = wp.tile([128, 4, E], BF16, tag="gW2")
            fsW1 = wp.tile([128, 2, NW, H], BF16, tag="fsW1")
            fsW2 = wp.tile([128, 4, NW, NL], BF16, tag="fsW2")
            gb1 = wp.tile([128, 4], F32, tag="gb1")
            fsb1 = wp.tile([128, NW, 4], F32, tag="fsb1")
            gb2b = wp.tile([128, E], F32, tag="gb2b")
            fsb2b = wp.tile([128, NW, NL], F32, tag="fsb2b")
            for t, d in [(gW1, gW1_d), (gW2, gW2_d), (fsW1, fsW1_d), (fsW2, fsW2_d),
                         (gb1, gb1_d), (fsb1, fsb1_d), (gb2b, gb2b_d), (fsb2b, fsb2b_d)]:
                nc.sync.dma_start(t[:], d[:])
            VT = wp.tile([128, 2, QT], BF16, tag="VT")
            DT = wp.tile([128, 2, QT], BF16, tag="DT")
            nc.sync.dma_start(VT[:], VT_d[:])
            nc.sync.dma_start(DT[:], DT_d[:])

            # ---- HBM scratch ----
            bp_reg = [dram.tile([BP_ELEMS], BF16, tag=f"bp{i}", name=f"bp{i}")
                      for i in range(NREG)]
            d_reg = [dram.tile([D_ELEMS], BF16, tag=f"dg{i}", name=f"dg{i}")
                     for i in range(NREG)]
            cinV = dram.tile([QT * E], BF16, tag="cinV")
            coutV = dram.tile([N * E], BF16, tag="coutV")
            cinW = dram.tile([NW * QB * 128 * NL], BF16, tag="cinW")
            coutW = dram.tile([NQ * NW * QB * 128 * NL], BF16, tag="coutW")

            def fap(t, offset, dims):
                a = t.tensor.ap()
                return bass.AP(a.tensor, offset, dims)

            def squeeze(ap):
                dims = [list(d) for d in ap.ap]
                kept = [dims[0]] + [d for d in dims[1:] if d[1] > 1]
                return bass.AP(ap.tensor, ap.offset, kept)

            # ---- zero the band regions (scatters only ever touch diagonals) ----
            zsb = stage.tile([128, 4096], BF16, tag="zero")
            nc.vector.memset(zsb[:], 0.0)
            ZC = 128 * 4096
            for reg, nelem in [(b, BP_ELEMS) for b in bp_reg] + [(d, D_ELEMS) for d in d_reg]:
                for o in range(0, nelem, ZC):
                    nc.sync.dma_start(fap(reg, o, [[4096, 128], [1, 4096]]), zsb[:])

            # ---- g MLP: V1q = gelu(V @ gW1 + gb1) @ gW2 + gb2 (own tokens) ----
            def mlp_hidden(rhsT, W1, b1col):
                """hidT [128, 4, QT] bf16 = gelu(W1.T @ rhsT + b1)."""
                hidT = stage.tile([128, 4, QT], BF16, tag="hidT")
                for ht in range(4):
                    for ns in range(0, QT, 512):
                        ps = psA.tile([128, 512], F32, tag="psA")
                        for ec in range(2):
                            nc.tensor.matmul(
                                ps[:],
                                W1[:, ec, ht * 128:(ht + 1) * 128],
                                rhsT[:, ec, ns:ns + 512],
                                start=(ec == 0), stop=(ec == 1),
                            )
                        nc.scalar.activation(
                            hidT[:, ht, ns:ns + 512], ps[:], AF.Gelu,
                            bias=b1col[:, ht:ht + 1],
                        )
                return hidT

            hidT = mlp_hidden(VT, gW1, gb1)
            V1q = stage.tile([128, QB, E], BF16, tag="V1q")
            for nb in range(QB):
                ps = psM.tile([128, E], F32, tag="psM")
                for ht in range(4):
                    nc.tensor.matmul(
                        ps[:], hidT[:, ht, nb * 128:(nb + 1) * 128], gW2[:, ht, :],
                        start=(ht == 0), stop=(ht == 3),
                    )
                nc.vector.tensor_add(V1q[:, nb, :], ps[:], gb2b[:])

            # AllGather V1 within each batch group -> full-batch V
            nc.sync.dma_start(
                fap(cinV, 0, [[E, 128], [128 * E, QB], [1, E]]), V1q[:]
            )
            nc.gpsimd.collective_compute(
                "AllGather", mybir.AluOpType.bypass,
                replica_groups=[[0, 1, 2, 3], [4, 5, 6, 7]],
                ins=[cinV.opt()], outs=[coutV.opt()],
            )
            Vcur = vp.tile([128, NB, E], BF16, tag="Vping")
            Vnxt = vp.tile([128, NB, E], BF16, tag="Vpong")
            nc.sync.dma_start(
                Vcur[:], fap(coutV, 0, [[E, 128], [128 * E, NB], [1, E]])
            )

            # ---- fs MLPs: W[k] for own tokens, all layers ----
            # Wmine layout [128 p, QB rr, NW k, NL l] so the cinW DMA merges
            # (k, l) into one contiguous dim (3-dim DMA AP limit).
            Wmine = wp.tile([128, QB, NW, NL], BF16, tag="Wmine")
            for k in range(NW):
                hk = mlp_hidden(DT, fsW1[:, :, k, :], fsb1[:, k, :])
                for nb in range(QB):
                    ps = psB.tile([128, NL], F32, tag="psB")
                    for ht in range(4):
                        nc.tensor.matmul(
                            ps[:], hk[:, ht, nb * 128:(nb + 1) * 128],
                            fsW2[:, ht, k, :],
                            start=(ht == 0), stop=(ht == 3),
                        )
                    nc.vector.tensor_add(Wmine[:, nb, k, :], ps[:], fsb2b[:, k, :])

            # AllGather W within batch group. cinW flat layout (rr, p, k, l).
            nc.sync.dma_start(
                fap(cinW, 0,
                    [[NW * NL, 128], [128 * NW * NL, QB], [1, NW * NL]]),
                Wmine[:],
            )
            nc.gpsimd.collective_compute(
                "AllGather", mybir.AluOpType.bypass,
                replica_groups=[[0, 1, 2, 3], [4, 5, 6, 7]],
                ins=[cinW.opt()], outs=[coutW.opt()],
            )
            # coutW flat layout (qq, rr, p, k, l); (qq, rr) merge to one dim.
            Wall = wp.tile([128, NW, NB, NL], BF16, tag="Wall")
            for k in range(NW):
                nc.sync.dma_start(
                    squeeze(Wall[:, k, :, :]),
                    fap(coutW, k * NL,
                        [[NW * NL, 128], [128 * NW * NL, NB], [1, NL]]),
                )

            # ---- scatter W onto band-matrix diagonals (one DMA per link;
            # strided dest costs an extra unit dim, so max 2 iter dims) ----
            def scatter_layer(k):
                bp, dg = bp_reg[k % NREG], d_reg[k % NREG]
                for l in range(9):  # offsets 0..128 -> band-pair matrices
                    nc.sync.dma_start(
                        fap(bp, OFFS[l] * 128, [[129, 128], [32768, NB]]),
                        squeeze(Wall[:, k, :, l:l + 1]),
                    )
                for j in range(4):  # offsets 256..2048 -> diag matrices
                    nc.sync.dma_start(
                        fap(dg, j * 16384, [[129, 128], [4 * 16384, NB]]),
                        squeeze(Wall[:, k, :, 9 + j:10 + j]),
                    )

            # ---- mixing layers ----
            outF = vp.tile([128, NB, E], F32, tag="outF")
            for k in range(NW):
                scatter_layer(k)
                bp, dg = bp_reg[k % NREG], d_reg[k % NREG]
                last = k == NW - 1
                for qq in range(NQ):
                    bsb = mats.tile([128, QB * 2, 128], BF16, tag="bsb")
                    dsb = mats.tile([128, QB * 4, 128], BF16, tag="dsb")
                    nc.sync.dma_start(
                        bsb[:],
                        fap(bp, qq * QB * 32768,
                            [[128, 128], [16384, QB * 2], [1, 128]]),
                    )
                    nc.sync.dma_start(
                        dsb[:],
                        fap(dg, qq * QB * 65536,
                            [[128, 128], [16384, QB * 4], [1, 128]]),
                    )
                    for rr in range(QB):
                        r = qq * QB + rr
                        ps = psM.tile([128, E], F32, tag="psM")
                        nc.tensor.matmul(ps[:], bsb[:, rr * 2, :],
                                         Vcur[:, r, :], start=True, stop=False)
                        nc.tensor.matmul(ps[:], bsb[:, rr * 2 + 1, :],
                                         Vcur[:, (r + 1) % NB, :],
                                         start=False, stop=False)
                        for j, cc in enumerate([2, 4, 8, 16]):
                            nc.tensor.matmul(ps[:], dsb[:, rr * 4 + j, :],
                                             Vcur[:, (r + cc) % NB, :],
                                             start=False, stop=(j == 3))
                        dst = outF[:, r, :] if last else Vnxt[:, r, :]
                        if r % 2 == 0:
                            nc.scalar.copy(dst, ps[:])
                        else:
                            nc.vector.tensor_copy(dst, ps[:])
                Vcur, Vnxt = Vnxt, Vcur

            nc.sync.dma_start(out_d[:], outF[:])
    nc.compile()
    return nc


def _get_compiled():
    if "nc" not in _CACHE:
        _CACHE["nc"] = _build()
    return _CACHE["nc"]


class _Runner:
    def __init__(self, nc):
        import jax
        import concourse.mybir as mybir
        from jax.sharding import Mesh, PartitionSpec
        from jax.experimental.shard_map import shard_map
        from concourse.bass2jax import (
            _bass_exec_p, install_neuronx_cc_hook, partition_id_tensor,
        )

        install_neuronx_cc_hook()
        self.jax = jax
        self.nc = nc
        in_names, out_names, out_avals, zero_outs = [], [], [], []
        partition_name = (
            nc.partition_id_tensor.name if nc.partition_id_tensor else None
        )
        for alloc in nc.m.functions[0].allocations:
            if not isinstance(alloc, mybir.MemoryLocationSet):
                continue
            name = alloc.memorylocations[0].name
            if alloc.kind == "ExternalInput":
                if name != partition_name:
                    in_names.append(name)
            elif alloc.kind == "ExternalOutput":
                out_names.append(name)
                out_avals.append(
                    jax.core.ShapedArray(
                        tuple(alloc.tensor_shape), mybir.dt.np(alloc.dtype)
                    )
                )
                zero_outs.append(
                    np.zeros(tuple(alloc.tensor_shape), mybir.dt.np(alloc.dtype))
                )
        self.in_names, self.out_names = list(in_names), out_names
        self.out_avals, self.zero_outs = out_avals, zero_outs
        n_params = len(in_names)
        all_in_names = in_names + out_names
        if partition_name is not None:
            all_in_names.append(partition_name)

        def _body(*args):
            operands = list(args)
            if partition_name is not None:
                operands.append(partition_id_tensor())
            return tuple(
                _bass_exec_p.bind(
                    *operands,
                    out_avals=tuple(out_avals),
                    in_names=tuple(all_in_names),
                    out_names=tuple(out_names),
                    lowering_input_output_aliases=(),
                    sim_require_finite=True,
                    sim_require_nnan=True,
                    nc=nc,
                )
            )

        devices = jax.devices()[:NCORES]
        mesh = Mesh(np.asarray(devices), ("core",))
        n_outs = len(out_names)
        self.fn = jax.jit(
            shard_map(
                _body, mesh=mesh,
                in_specs=(PartitionSpec("core"),) * (n_params + n_outs),
                out_specs=(PartitionSpec("core"),) * n_outs,
                check_rep=False,
            ),
            donate_argnums=tuple(range(n_params, n_params + n_outs)),
            keep_unused=True,
        )

    def stage_inputs(self, in_maps):
        import jax
        concat = [
            np.concatenate([np.asarray(in_maps[c][n]) for c in range(NCORES)], axis=0)
            for n in self.in_names
        ]
        return [jax.device_put(a) for a in concat]

    def fresh_zeros(self):
        import jax
        return [
            jax.device_put(np.zeros((NCORES * z.shape[0], *z.shape[1:]), z.dtype))
            for z in self.zero_outs
        ]

    def run(self, staged):
        outs = self.fn(*staged, *self.fresh_zeros())
        return [o.block_until_ready() for o in outs]


def _get_run():
    if "runner" not in _CACHE or _CACHE.get("runner") is None:
        _CACHE["runner"] = _Runner(_get_compiled())
    return _CACHE["runner"]


def kernel(**inputs) -> np.ndarray:
    in_maps = _host_prep(
        inputs["V"], inputs["data"], inputs["gW1"], inputs["gb1"], inputs["gW2"],
        inputs["gb2"], inputs["fsW1"], inputs["fsb1"], inputs["fsW2"],
        inputs["fsb2"],
    )
    r = _get_run()
    outs = r.run(r.stage_inputs(in_maps))
    # outs[i] has shape [NCORES*128, NB, E]; core c rows [128c, 128c+128)
    oidx = r.out_names.index("out")
    arr = np.asarray(outs[oidx])
    out = np.empty((B, N, E), np.float32)
    for b in range(B):
        o = arr[b * NQ * 128:(b * NQ + 1) * 128]  # [128, NB, E]
        out[b] = o.transpose(1, 0, 2).reshape(N, E)
    return out
